# revision 2
# baseline (speedup 1.0000x reference)
"""Trainium2 Bass kernel for segment_reduce MLP (nn_HeadSemantic_35983236006251).

Math shortcut: Linear commutes with segment_sum, so
    pooled = segment_sum(x @ W_in + b_in) = segment_sum(x) @ W_in + counts * b_in
and the kernel reduces to memory-bound streaming of x into per-segment sums,
followed by a tiny MLP on [4096, 256].

v2 design (vs. the one-hot baseline):
  * x is streamed in fp8 (e4m3) instead of fp32 -- 4x less HBM traffic.
    Host-side quantization uses error feedback (sigma-delta) along each
    (segment, column) chain, so the device's exact-fp32 PSUM accumulation sees
    a segment-sum error of ~1 quantization step instead of ~sqrt(n) steps.
    Measured end-to-end rel err ~4e-3 (gate is 2e-2).
  * No per-tile one-hot build at all: segments are sorted by size on the host
    and assigned one-per-partition; x is re-laid-out in DRAM as per-partition
    row streams.  The segment-sum is then a PSUM accumulation with a CONSTANT
    doubled-identity lhsT in fp8 DoubleRow mode (2 tiles of 128 rows per
    matmul at 0.5 cycles/row).  Padding cost of the sorted layout is ~6%.
  * x is DMAed in big per-partition-contiguous slabs (8 KB/partition), so the
    whole stream is ~30 DMA instructions instead of ~1000 (the baseline paid
    ~565 ns of SP sequencer time per dma_start).
  * The MLP runs per 128-segment window as soon as that window's pooled sums
    flush, overlapped with streaming of later windows; weights/activations in
    bf16 (PSUM accumulation fp32), biases+counts in fp32.

Sharding: segments sorted by size desc; window w (of 32) = segments
[128w, 128w+128); slot-group s = windows [8s, 8s+8); core c takes window
8s + c of each group.  All cores share one SPMD program whose per-slot tile
counts are the group maxima.
"""

import sys
import numpy as np
import ml_dtypes
from contextlib import ExitStack

sys.path.insert(0, "/opt/trn_rl_repo")

import concourse.bass as bass
from concourse import mybir
from concourse.bass_utils import run_bass_kernel_spmd

N = 1_000_000
D = 256
NSEG = 4096
N_CORES = 8
NSLOT = 4                  # windows per core
SEG = NSLOT * 128          # segments per core
F32 = mybir.dt.float32
BF16 = mybir.dt.bfloat16
F8 = mybir.dt.float8e4
NPF8 = ml_dtypes.float8_e4m3
NPBF = ml_dtypes.bfloat16
SLAB_PAIRS = 16            # row-tile pairs per DMA slab (32 tiles, 8KB/part)
RING = 10                  # x slab ring slots
DR = mybir.MatmulPerfMode.DoubleRow


def _slab_plan(TP):
    """TP = tiles per slot (even).  Returns (slabs, cumslabs) where each slab
    is (slot, dram_tile0, npairs, first_of_slot)."""
    slabs = []
    cumslabs = []
    base = 0
    for s, tp in enumerate(TP):
        pairs = tp // 2
        k = 0
        while k < pairs:
            np_ = min(SLAB_PAIRS, pairs - k)
            slabs.append((s, base + 2 * k, np_, k == 0))
            k += np_
        cumslabs.append(len(slabs))
        base += tp
    return slabs, cumslabs


def build_program(TP):
    nc = bass.Bass()
    TOT = sum(TP)
    PAIRS = [tp // 2 for tp in TP]
    slabs, cumslabs = _slab_plan(TP)

    xdr_in = nc.declare_dram_parameter("xdr", [128, TOT, D], F8, False)
    id2_in = nc.declare_dram_parameter("id2", [128, 2, 128], F8, False)
    idf_in = nc.declare_dram_parameter("idf", [128, 128], F32, False)
    win_in = nc.declare_dram_parameter("win", [D, D], BF16, False)
    w1_in = nc.declare_dram_parameter("w1", [D, 2 * D], BF16, False)
    w2_in = nc.declare_dram_parameter("w2", [2 * D, D], BF16, False)
    bin_in = nc.declare_dram_parameter("bin", [1, D], F32, False)
    b1_in = nc.declare_dram_parameter("b1", [1, 2 * D], F32, False)
    b2_in = nc.declare_dram_parameter("b2", [1, D], F32, False)
    ones_in = nc.declare_dram_parameter("ones", [1, SEG], F32, False)
    crow_in = nc.declare_dram_parameter("crow", [1, SEG], F32, False)
    outT_ext = nc.declare_dram_parameter("outT", [D, SEG], F32, True)

    NMLPC = 9  # win,w1,w2 tiles loaded as 8 DMAs + bin,b1,b2,ones,crow -> see below

    with ExitStack() as es:
        def sem(name):
            return es.enter_context(nc.semaphore(name))

        def sb(name, shape, dt):
            return es.enter_context(nc.sbuf_tensor(name, shape, dt))

        def psum(name, shape, dt):
            return es.enter_context(nc.psum_tensor(name, shape, dt))

        s_x, s_cc, s_c, s_pe, s_fl = sem("x"), sem("cc"), sem("c"), sem("pe"), sem("fl")
        s_tr, s_ptc, s_z, s_zc = sem("tr"), sem("ptc"), sem("z"), sem("zc")
        s_h, s_hc, s_o, s_oc, s_do = sem("h"), sem("hc"), sem("o"), sem("oc"), sem("do")

        id2_sb = sb("id2_sb", [128, 2, 128], F8)
        idf_sb = sb("idf_sb", [128, 128], F32)
        xbuf = [sb(f"xb{i}", [128, 2 * SLAB_PAIRS, D], F8) for i in range(RING)]
        winkb = [sb(f"wink{k}", [128, D], BF16) for k in range(2)]
        w1kb = [sb(f"w1k{k}", [128, 2 * D], BF16) for k in range(2)]
        w2kb = [sb(f"w2k{k}", [128, D], BF16) for k in range(4)]
        bin_sb = sb("bin_sb", [1, D], F32)
        b1_sb = sb("b1_sb", [1, 2 * D], F32)
        b2_sb = sb("b2_sb", [1, D], F32)
        ones_sb = sb("ones_sb", [1, SEG], F32)
        crow_sb = sb("crow_sb", [1, SEG], F32)
        po = [sb(f"po{w}", [128, D], F32) for w in range(NSLOT)]
        pT = [sb(f"pT{k}", [128, SEG], BF16) for k in range(2)]
        zT = [sb(f"zT{k}", [128, SEG], BF16) for k in range(2)]
        hT = [sb(f"hT{j}", [128, SEG], BF16) for j in range(4)]
        ot = [sb(f"ot{j}", [128, SEG], F32) for j in range(2)]

        pb = [psum("pb0", [128, 512], F32), psum("pb1", [128, 512], F32)]
        trP = psum("trP", [128, 512], F32)
        zP = psum("zP", [128, 512], F32)
        hP = [psum("hP0", [128, 512], F32), psum("hP1", [128, 512], F32)]
        oP = psum("oP", [128, 512], F32)

        NSLAB = len(slabs)

        with nc.Block() as block:

            @block.sync
            def _(sp):
                # pure x streaming with ring recycling
                for g, (s, t0, np_, first) in enumerate(slabs):
                    if g >= RING:
                        sp.wait_ge(s_pe, g - RING + 1)
                    sp.dma_start(out=xbuf[g % RING][:, 0:2 * np_, :],
                                 in_=xdr_in[:, t0:t0 + 2 * np_, :]
                                 ).then_inc(s_x, 16)

            @block.scalar
            def _(a):
                # constants: PE-startup ones first, then MLP consts, then outputs
                a.dma_start(out=id2_sb[:, :, :], in_=id2_in[:, :, :]).then_inc(s_cc, 16)
                a.dma_start(out=idf_sb[:, :], in_=idf_in[:, :]).then_inc(s_cc, 16)
                for k in range(2):
                    a.dma_start(out=winkb[k][:, :],
                                in_=win_in[k * 128:(k + 1) * 128, :]).then_inc(s_c, 16)
                for k in range(2):
                    a.dma_start(out=w1kb[k][:, :],
                                in_=w1_in[k * 128:(k + 1) * 128, :]).then_inc(s_c, 16)
                for k in range(4):
                    a.dma_start(out=w2kb[k][:, :],
                                in_=w2_in[k * 128:(k + 1) * 128, :]).then_inc(s_c, 16)
                a.dma_start(out=bin_sb[:, :], in_=bin_in[:, :]).then_inc(s_c, 16)
                a.dma_start(out=b1_sb[:, :], in_=b1_in[:, :]).then_inc(s_c, 16)
                a.dma_start(out=b2_sb[:, :], in_=b2_in[:, :]).then_inc(s_c, 16)
                a.dma_start(out=ones_sb[:, :], in_=ones_in[:, :]).then_inc(s_c, 16)
                a.dma_start(out=crow_sb[:, :], in_=crow_in[:, :]).then_inc(s_c, 16)
                # outputs, per window as they complete
                for w in range(NSLOT):
                    for j in range(2):
                        a.wait_ge(s_oc, 2 * w + j + 1)
                        a.dma_start(
                            out=outT_ext[j * 128:(j + 1) * 128, w * 128:(w + 1) * 128],
                            in_=ot[j][:, w * 128:(w + 1) * 128]).then_inc(s_do, 16)
                a.wait_ge(s_do, 16 * 2 * NSLOT)

            def pe_mlp(pe, w):
                par = w % 2
                wc = slice(128 * w, 128 * (w + 1))
                hf = par * 256
                pe.wait_ge(s_fl, w + 1)
                if w >= 2:
                    pe.wait_ge(s_ptc, 2 * (w - 1))
                for k in range(2):
                    pe.transpose(trP[:, hf + k * 128:hf + (k + 1) * 128],
                                 po[w][:, k * 128:(k + 1) * 128],
                                 idf_sb[:, :]).then_inc(s_tr, 1)
                # ---- z = pooled @ W_in + counts * b_in ----
                pe.wait_ge(s_ptc, 2 * w + 2)
                if w == 0:
                    pe.wait_ge(s_c, 16 * 13)
                if w >= 2:
                    pe.wait_ge(s_zc, 2 * (w - 1))
                for j in range(2):
                    jc = slice(j * 128, (j + 1) * 128)
                    dst = zP[:, hf + j * 128:hf + (j + 1) * 128]
                    pe.matmul(dst, winkb[0][:, jc], pT[0][:, wc], start=True, stop=False)
                    pe.matmul(dst, winkb[1][:, jc], pT[1][:, wc], start=False, stop=False)
                    pe.matmul(dst, bin_sb[0:1, jc], crow_sb[0:1, wc],
                              start=False, stop=True).then_inc(s_z, 1)
                # ---- h = relu(z @ W1 + b1) ----
                pe.wait_ge(s_zc, 2 * w + 2)
                if w >= 2:
                    pe.wait_ge(s_hc, 4 * (w - 1))
                for j in range(4):
                    jc = slice(j * 128, (j + 1) * 128)
                    dst = hP[par][:, j * 128:(j + 1) * 128]
                    pe.matmul(dst, w1kb[0][:, jc], zT[0][:, wc], start=True, stop=False)
                    pe.matmul(dst, w1kb[1][:, jc], zT[1][:, wc], start=False, stop=False)
                    pe.matmul(dst, b1_sb[0:1, jc], ones_sb[0:1, wc],
                              start=False, stop=True).then_inc(s_h, 1)
                # ---- o = h @ W2 + b2 ----
                pe.wait_ge(s_hc, 4 * w + 4)
                if w >= 2:
                    pe.wait_ge(s_oc, 2 * (w - 1))
                for j in range(2):
                    jc = slice(j * 128, (j + 1) * 128)
                    dst = oP[:, hf + j * 128:hf + (j + 1) * 128]
                    for i in range(4):
                        pe.matmul(dst, w2kb[i][:, jc], hT[i][:, wc],
                                  start=(i == 0), stop=False)
                    pe.matmul(dst, b2_sb[0:1, jc], ones_sb[0:1, wc],
                              start=False, stop=True).then_inc(s_o, 1)

            @block.tensor
            def _(pe):
                pe.wait_ge(s_cc, 32)
                # index of the slab after which to emit MLP(s-1): one full slab
                # into slot s, so the flush of s-1 has had time to complete.
                mlp_after = {}
                for g, (s, t0, np_, first) in enumerate(slabs):
                    if first and s >= 1:
                        mlp_after[min(g + 1, NSLAB - 1)] = s - 1
                for g, (s, t0, np_, first) in enumerate(slabs):
                    pe.wait_ge(s_x, 16 * (g + 1))
                    if first and s >= 2:
                        pe.wait_ge(s_fl, s - 1)
                    pair0 = sum(PAIRS[:s])  # unused; pairs indexed within slot
                    k0 = (t0 - sum(TP[:s])) // 2
                    for i in range(np_):
                        kk = k0 + i
                        mm = pe.matmul(pb[s % 2][:, 0:D], id2_sb[:, :, :],
                                       xbuf[g % RING][:, 2 * i:2 * i + 2, :],
                                       start=(kk == 0), stop=(kk == PAIRS[s] - 1),
                                       perf_mode=DR)
                        if i == np_ - 1:
                            mm.then_inc(s_pe, 1)
                    if g in mlp_after:
                        pe_mlp(pe, mlp_after[g])
                pe_mlp(pe, NSLOT - 1)

            @block.vector
            def _(v):
                for w in range(NSLOT):
                    par = w % 2
                    # flush pooled sums of window w
                    v.wait_ge(s_pe, cumslabs[w])
                    v.tensor_copy(po[w][:, :], pb[par][:, 0:D]).then_inc(s_fl, 1)
                    if w >= 1:
                        mlp_copies(v, w - 1)
                mlp_copies(v, NSLOT - 1)

            def mlp_copies(v, w):
                par = w % 2
                wc = slice(128 * w, 128 * (w + 1))
                hf = par * 256
                v.wait_ge(s_tr, 2 * w + 2)
                for k in range(2):
                    v.tensor_copy(pT[k][:, wc],
                                  trP[:, hf + k * 128:hf + (k + 1) * 128]
                                  ).then_inc(s_ptc, 1)
                for j in range(2):
                    v.wait_ge(s_z, 2 * w + j + 1)
                    v.tensor_copy(zT[j][:, wc],
                                  zP[:, hf + j * 128:hf + (j + 1) * 128]
                                  ).then_inc(s_zc, 1)
                for j in range(4):
                    v.wait_ge(s_h, 4 * w + j + 1)
                    v.tensor_relu(hT[j][:, wc],
                                  hP[par][:, j * 128:(j + 1) * 128]).then_inc(s_hc, 1)
                for j in range(2):
                    v.wait_ge(s_o, 2 * w + j + 1)
                    v.tensor_copy(ot[j][:, wc],
                                  oP[:, hf + j * 128:hf + (j + 1) * 128]
                                  ).then_inc(s_oc, 1)

    return nc


def _quantize_feedback(x, sizes, starts, order):
    """fp8 e4m3 with per-(segment, column) sigma-delta error feedback."""
    xq = np.empty(x.shape, dtype=NPF8)
    # process segments in descending-size order so live set is a prefix
    sz_d = sizes[order]                       # descending
    st_d = starts[order]
    carry = np.zeros((NSEG, D), np.float32)
    maxlen = int(sz_d[0])
    for r in range(maxlen):
        m = int(np.searchsorted(-sz_d, -(r + 1), side="right"))
        rows = st_d[:m] + r
        acc = x[rows] + carry[:m]
        q = acc.astype(NPF8)
        xq[rows] = q
        carry[:m] = acc - q.astype(np.float32)
    return xq


def _plan(batch):
    sizes = np.bincount(batch, minlength=NSEG).astype(np.int64)
    starts = np.concatenate([[0], np.cumsum(sizes)])[:-1]
    order = np.argsort(-sizes, kind="stable")
    TP = [int(sizes[order[1024 * s]] + 1) // 2 * 2 for s in range(NSLOT)]
    return sizes, starts, order, TP


def kernel(**inputs):
    x = np.ascontiguousarray(np.asarray(inputs["x"], np.float32))
    batch = np.asarray(inputs["batch"]).astype(np.int64)
    W_in = np.asarray(inputs["W_in"], np.float32)
    b_in = np.asarray(inputs["b_in"], np.float32).reshape(1, D)
    W1 = np.asarray(inputs["W1"], np.float32)
    b1 = np.asarray(inputs["b1"], np.float32).reshape(1, 2 * D)
    W2 = np.asarray(inputs["W2"], np.float32)
    b2 = np.asarray(inputs["b2"], np.float32).reshape(1, D)

    sizes, starts, order, TP = _plan(batch)
    TOT = sum(TP)
    xq = _quantize_feedback(x, sizes, starts, order)
    xq_pad = np.concatenate([xq, np.zeros((1, D), NPF8)])

    id2 = np.stack([np.eye(128, dtype=np.float32)] * 2, axis=1).astype(NPF8)
    idf = np.eye(128, dtype=np.float32)
    shared = dict(
        id2=id2, idf=idf,
        win=W_in.astype(NPBF), w1=W1.astype(NPBF), w2=W2.astype(NPBF),
        bin=b_in, b1=b1, b2=b2, ones=np.ones((1, SEG), np.float32),
    )

    per_core = []
    core_segs = []
    for c in range(N_CORES):
        idx = np.full((128, TOT), N, np.int64)
        segs_c = np.empty(SEG, np.int64)
        off = 0
        for s in range(NSLOT):
            w = 8 * s + c
            segs = order[128 * w:128 * w + 128]
            segs_c[128 * s:128 * s + 128] = segs
            for p in range(128):
                n = int(sizes[segs[p]])
                idx[p, off:off + n] = starts[segs[p]] + np.arange(n)
            off += TP[s]
        xdr = xq_pad[idx.reshape(-1)].reshape(128, TOT, D)
        crow = sizes[segs_c].astype(np.float32).reshape(1, SEG)
        m = dict(shared)
        m.update(xdr=xdr, crow=crow)
        per_core.append(m)
        core_segs.append(segs_c)

    nc = build_program(TP)
    res = run_bass_kernel_spmd(nc, per_core, list(range(N_CORES)))

    out = np.empty((NSEG, D), np.float32)
    for c in range(N_CORES):
        out[core_segs[c]] = res.results[c]["outT"].T
    return out


# revision 46
# speedup vs baseline: 7.4054x; 7.4054x over previous
"""Trainium2 Bass kernel for segment_reduce MLP (nn_HeadSemantic_35983236006251).

Math shortcut: Linear commutes with segment_sum, so
    pooled = segment_sum(x @ W_in + b_in) = segment_sum(x) @ W_in + counts * b_in
and the kernel reduces to memory-bound streaming of x into per-segment sums,
followed by a tiny MLP on [4096, 256].

Design (vs. the one-hot baseline):
  * x is streamed in fp8 (e4m3) instead of fp32 -- 4x less HBM traffic.
    Host-side quantization uses error feedback (sigma-delta) along each
    (segment, column) chain, so the device's exact-fp32 PSUM accumulation sees
    a segment-sum error of ~1 quantization step instead of ~sqrt(n) steps.
    Measured end-to-end rel err ~4e-3 (gate is 2e-2).
  * No per-tile one-hot build at all: segments are sorted by size on the host
    and assigned one-per-partition; x is re-laid-out in DRAM as per-partition
    row streams.  The segment-sum is then a PSUM accumulation with a CONSTANT
    doubled-identity lhsT in fp8 DoubleRow mode (2 tiles of 128 rows per
    matmul).
  * x is DMAed in big per-partition-contiguous slabs (16 KB/partition),
    round-robined over THREE DGE queues (SP / Activation / Pool) so the
    descriptor rings never throttle the HBM stream; DVE's queue carries the
    small constants and drains results.
  * The MLP runs per 64-segment chunk as soon as its window's pooled sums
    flush, overlapped with streaming of later windows; weights/activations in
    bf16 (PSUM accumulation fp32); biases fused into the DVE PSUM->SBUF
    copies (per-partition tensor_scalar add / add+relu), the counts*b_in term
    via scalar_tensor_tensor against a host-broadcast counts plane.  PSUM
    scratch alternates by chunk parity so chunk q+1's matmuls overlap chunk
    q's drain copies; only the final chunk's drain is tail latency.

Sharding: segments sorted by size desc; window w (of 32) = segments
[128w, 128w+128); slot-group s = windows [8s, 8s+8); core c takes window
8s + c of each group.  All cores share one SPMD program whose per-slot tile
counts are the group maxima.
"""

import sys
import numpy as np
import ml_dtypes
from contextlib import ExitStack

sys.path.insert(0, "/opt/trn_rl_repo")

import concourse.bass as bass
from concourse import mybir
from concourse.bass_utils import run_bass_kernel_spmd

N = 1_000_000
D = 256
NSEG = 4096
N_CORES = 8
NSLOT = 4                  # windows per core
SEG = NSLOT * 128          # segments per core
NCHUNK = 2 * NSLOT         # 64-segment MLP chunks
F32 = mybir.dt.float32
BF16 = mybir.dt.bfloat16
F8 = mybir.dt.float8e4
NPF8 = ml_dtypes.float8_e4m3
NPBF = ml_dtypes.bfloat16
SLAB_PAIRS = 32            # row-tile pairs per DMA slab (64 tiles, 16KB/part)
NQ = 4                     # x DMA queues (SP, Act, Pool, DVE-prolog)
QSLOTS = [[0, 1, 2], [3, 4, 5], [6, 7], [8, 9]]   # queue-exclusive xbuf slots
QCAP = [99, 99, 99, 2]     # DVE only issues 2 slabs, before its drain loop
NRING = 10
DR = mybir.MatmulPerfMode.DoubleRow
ADD = mybir.AluOpType.add
MAX = mybir.AluOpType.max
MULT = mybir.AluOpType.mult
NCONST = 12                # MLP const DMAs on s_c


def _slab_plan(TP):
    """TP = tiles per slot (even).  Each slab is
    (slot, dram_tile0, npairs, first_of_slot)."""
    slabs = []
    cumslabs = []
    base = 0
    for s, tp in enumerate(TP):
        pairs = tp // 2
        k = 0
        while k < pairs:
            np_ = min(SLAB_PAIRS, pairs - k)
            slabs.append((s, base + 2 * k, np_, k == 0))
            k += np_
        cumslabs.append(len(slabs))
        base += tp
    return slabs, cumslabs


def build_program(TP):
    nc = bass.Bass()
    TOT = sum(TP)
    PAIRS = [tp // 2 for tp in TP]
    slabs, cumslabs = _slab_plan(TP)
    NSLAB = len(slabs)

    xdr_in = nc.declare_dram_parameter("xdr", [128, TOT, D], F8, False)
    id2_in = nc.declare_dram_parameter("id2", [128, 2, 128], F8, False)
    idf_in = nc.declare_dram_parameter("idf", [128, 128], F32, False)
    win_in = nc.declare_dram_parameter("win", [D, D], BF16, False)
    w1_in = nc.declare_dram_parameter("w1", [D, 2 * D], BF16, False)
    w2_in = nc.declare_dram_parameter("w2", [2 * D, D], BF16, False)
    binT_in = nc.declare_dram_parameter("binT", [128, 2], F32, False)
    b1T_in = nc.declare_dram_parameter("b1T", [128, 4], F32, False)
    b2T_in = nc.declare_dram_parameter("b2T", [128, 2], F32, False)
    cbc_in = nc.declare_dram_parameter("cbc", [128, SEG], F32, False)
    outT_ext = nc.declare_dram_parameter("outT", [D, SEG], F32, True)

    with ExitStack() as es:
        def sem(name):
            return es.enter_context(nc.semaphore(name))

        def sb(name, shape, dt):
            return es.enter_context(nc.sbuf_tensor(name, shape, dt))

        def psum(name, shape, dt):
            return es.enter_context(nc.psum_tensor(name, shape, dt))

        s_cc, s_cf, s_pe, s_fl = sem("cc"), sem("cf"), sem("pe"), sem("fl")
        s_ca, s_cb, s_cd, s_ce = sem("ca"), sem("cb"), sem("cd"), sem("ce")
        s_tr, s_ptc, s_z, s_zc = sem("tr"), sem("ptc"), sem("z"), sem("zc")
        s_h, s_hc, s_o, s_oc = sem("h"), sem("hc"), sem("o"), sem("oc")
        s_do, s_do2 = sem("do"), sem("do2")
        s_x = [sem(f"x{i}") for i in range(RING)]

        id2_sb = sb("id2_sb", [128, 2, 128], F8)
        idf_sb = sb("idf_sb", [128, 128], F32)
        xbuf = [sb(f"xb{i}", [128, 2 * SLAB_PAIRS, D], F8) for i in range(RING)]
        winkb = [sb(f"wink{k}", [128, D], BF16) for k in range(2)]
        w1kb = [sb(f"w1k{k}", [128, 2 * D], BF16) for k in range(2)]
        w2kb = [sb(f"w2k{k}", [128, D], BF16) for k in range(4)]
        binT = sb("binT_sb", [128, 2], F32)
        b1T = sb("b1T_sb", [128, 4], F32)
        b2T = sb("b2T_sb", [128, 2], F32)
        cbc = sb("cbc_sb", [128, SEG], F32)
        po = [sb(f"po{w}", [128, D], F32) for w in range(NSLOT)]
        pT = [sb(f"pT{k}", [128, SEG], BF16) for k in range(2)]
        zT = [sb(f"zT{k}", [128, SEG], BF16) for k in range(2)]
        hT = [sb(f"hT{j}", [128, SEG], BF16) for j in range(4)]
        ot = [sb(f"ot{j}", [128, SEG], F32) for j in range(2)]

        # 7 PSUM banks.  A[h]: per chunk parity -- cols 0:128 transposes,
        # 128:256 z, 256:384 o.  hB[h]: h-stage.  pb: stream accumulator
        # ring of 3, so a slot's first matmul never waits on the previous
        # slot's flush.
        pb = [psum(f"pb{i}", [128, D], F32) for i in range(3)]
        A = [psum("A0", [128, 512], F32), psum("A1", [128, 512], F32)]
        hB = [psum("hB0", [128, D], F32), psum("hB1", [128, D], F32)]

        # x slab -> queue: greedy by estimated queue finish time, so slabs
        # arrive roughly in consumption order despite Pool's const preamble.
        # Ring slots are queue-exclusive (SWDGE sems must be Pool-private):
        # queue qi owns slots [qi*RING//NQ, (qi+1)*RING//NQ).
        qload = [700.0, 0.0, 6800.0]           # SP (id2 first), Act, Pool
        queue_of = []
        for (s, t0, np_, first) in slabs:
            qi = min(range(NQ), key=lambda i: qload[i])
            queue_of.append(qi)
            qload[qi] += np_ * 512 * 0.386 + 120
        del qload
        RPQ = RING // NQ
        slot_of = [0] * NSLAB
        use_of = [0] * NSLAB      # how many times this slot was used before
        prev_user = [0] * NSLAB   # global index of the slot's previous user
        _count = {}
        _last = {}
        for g, qi in enumerate(queue_of):
            k = _count.get(qi, 0)
            slot = qi * RPQ + (k % RPQ)
            slot_of[g] = slot
            use_of[g] = k // RPQ
            prev_user[g] = _last.get(slot, -1)
            _last[slot] = g
            _count[qi] = k + 1

        def stream_queue(eng, qi):
            for g in range(NSLAB):
                if queue_of[g] != qi:
                    continue
                s, t0, np_, first = slabs[g]
                if use_of[g]:
                    eng.wait_ge(s_x[slot_of[g]], 16 * use_of[g])
                    eng.wait_ge(s_pe, prev_user[g] + 1)
                eng.dma_start(out=xbuf[slot_of[g]][:, 0:2 * np_, :],
                              in_=xdr_in[:, t0:t0 + 2 * np_, :]
                              ).then_inc(s_x[slot_of[g]], 16)

        with nc.Block(no_gpsimd_drain=True) as block:

            def out_dmas(eng, j, dsem):
                # per-chunk outputs for feature half j, chained on dsem
                for q in range(NCHUNK):
                    w, h = divmod(q, 2)
                    wch = slice(128 * w + 64 * h, 128 * w + 64 * h + 64)
                    eng.wait_ge(s_oc, 2 * q + j + 1)
                    if q:
                        eng.wait_ge(dsem, 16 * q)
                    eng.dma_start(out=outT_ext[j * 128:(j + 1) * 128, wch],
                                  in_=ot[j][:, wch]).then_inc(dsem, 16)
                eng.wait_ge(dsem, 16 * NCHUNK)

            @block.sync
            def _(sp):
                sp.dma_start(out=id2_sb[:, :, :], in_=id2_in[:, :, :]
                             ).then_inc(s_cc, 16)
                stream_queue(sp, 0)
                out_dmas(sp, 0, s_do)

            @block.scalar
            def _(a):
                stream_queue(a, 1)
                out_dmas(a, 1, s_do2)

            @block.gpsimd
            def _(gp):
                # small constants first: independent same-sem chains,
                # interleaved so each link's wait is satisfied on arrival.
                chains = {
                    s_ca: [(winkb[k][:, :], win_in[k * 128:(k + 1) * 128, :])
                           for k in range(2)],
                    s_cb: [(w1kb[k][:, :], w1_in[k * 128:(k + 1) * 128, :])
                           for k in range(2)],
                    s_cd: [(w2kb[k][:, :], w2_in[k * 128:(k + 1) * 128, :])
                           for k in range(4)],
                    s_ce: [(binT[:, :], binT_in[:, :]), (b1T[:, :], b1T_in[:, :]),
                           (b2T[:, :], b2T_in[:, :]), (cbc[:, :], cbc_in[:, :])],
                    s_cf: [(idf_sb[:, :], idf_in[:, :])],
                }
                depth = {}
                for rnd in range(4):
                    for cs, lst in chains.items():
                        if rnd < len(lst):
                            k = depth.get(cs, 0)
                            if k:
                                gp.wait_ge(cs, 16 * k)
                            dst, src = lst[rnd]
                            gp.dma_start(out=dst, in_=src).then_inc(cs, 16)
                            depth[cs] = k + 1
                stream_queue(gp, 2)

            # ---- PE-side MLP stages for chunk (w, h): 64 segment columns.
            # Stages are emitted one slab apart so every wait on a DVE drain
            # is satisfied before PE reaches it (no streaming stalls).
            def _chunk(w, h):
                q = 2 * w + h
                return (q, q % 2,
                        slice(128 * w + 64 * h, 128 * w + 64 * h + 64))

            def mlp_tr(pe, w, h):
                q, cp, wch = _chunk(w, h)
                hsl = slice(64 * h, 64 * h + 64)
                if q == 0:
                    pe.wait_ge(s_cf, 16)   # idf loaded
                if q >= 2:
                    # chunk-parity PSUM reuse: chunk q-2's drains must be done
                    pe.wait_ge(s_ptc, 2 * (q - 1))
                    pe.wait_ge(s_zc, 2 * (q - 1))
                    pe.wait_ge(s_oc, 2 * (q - 1))
                for k in range(2):
                    pe.wait_ge(s_fl, 2 * w + k + 1)
                    pe.transpose(A[cp][:, k * 64:(k + 1) * 64],
                                 po[w][hsl, k * 128:(k + 1) * 128],
                                 idf_sb[hsl, hsl]).then_inc(s_tr, 1)

            def mlp_z(pe, w, h):
                # z = pooled @ W_in  (counts*b_in fused into DVE drain)
                q, cp, wch = _chunk(w, h)
                pe.wait_ge(s_ptc, 2 * q + 2)
                if q == 0:
                    pe.wait_ge(s_ca, 32)
                for j in range(2):
                    jc = slice(j * 128, (j + 1) * 128)
                    dst = A[cp][:, 128 + j * 64:128 + (j + 1) * 64]
                    pe.matmul(dst, winkb[0][:, jc], pT[0][:, wch], start=True, stop=False)
                    pe.matmul(dst, winkb[1][:, jc], pT[1][:, wch],
                              start=False, stop=True).then_inc(s_z, 1)

            def mlp_h(pe, w, h):
                # h = relu(z @ W1 + b1)  (bias+relu fused into DVE drain)
                q, cp, wch = _chunk(w, h)
                pe.wait_ge(s_zc, 2 * q + 2)
                if q == 0:
                    pe.wait_ge(s_cb, 32)
                if q >= 2:
                    pe.wait_ge(s_hc, 4 * (q - 1))
                for j in range(4):
                    jc = slice(j * 128, (j + 1) * 128)
                    dst = hB[cp][:, j * 64:(j + 1) * 64]
                    pe.matmul(dst, w1kb[0][:, jc], zT[0][:, wch], start=True, stop=False)
                    pe.matmul(dst, w1kb[1][:, jc], zT[1][:, wch],
                              start=False, stop=True).then_inc(s_h, 1)

            def mlp_o(pe, w, h):
                # o = h @ W2  (b2 fused into DVE drain)
                q, cp, wch = _chunk(w, h)
                pe.wait_ge(s_hc, 4 * q + 4)
                if q == 0:
                    pe.wait_ge(s_cd, 64)
                for j in range(2):
                    jc = slice(j * 128, (j + 1) * 128)
                    dst = A[cp][:, 256 + j * 64:256 + (j + 1) * 64]
                    for i in range(4):
                        mm = pe.matmul(dst, w2kb[i][:, jc], hT[i][:, wch],
                                       start=(i == 0), stop=(i == 3))
                    mm.then_inc(s_o, 1)

            @block.tensor
            def _(pe):
                pe.wait_ge(s_cc, 16)   # id2 loaded
                # clock warm-up: the PE p-state ramps to full speed only after
                # 3us of continuous execution; burn idle pre-stream time on
                # dummy matmuls so slab 0 is processed at full rate.
                for _ in range(100):
                    pe.matmul(hB[1][:, 0:128], id2_sb[:, :, :], id2_sb[:, :, :],
                              start=True, stop=True, perf_mode=DR)
                # stage schedule: window s-1's MLP stages spread over the
                # first 4 slabs of slot s; o(s-1, 1) lands after the next
                # slot's first slab so its s_hc wait is pre-satisfied.
                stage_after = {}
                for g, (s, t0, np_, first) in enumerate(slabs):
                    if first and s >= 1:
                        last = cumslabs[s] - 1
                        p = s - 1
                        stage_after.setdefault(g, []).append((mlp_tr, p, 0))
                        stage_after.setdefault(min(g + 1, last), []).extend(
                            [(mlp_z, p, 0), (mlp_tr, p, 1)])
                        stage_after.setdefault(min(g + 2, last), []).extend(
                            [(mlp_h, p, 0), (mlp_z, p, 1)])
                        stage_after.setdefault(min(g + 3, last), []).extend(
                            [(mlp_o, p, 0), (mlp_h, p, 1), (mlp_o, p, 1)])
                for g, (s, t0, np_, first) in enumerate(slabs):
                    pe.wait_ge(s_x[slot_of[g]], 16 * (use_of[g] + 1))
                    if first and s >= 3:
                        pe.wait_ge(s_fl, 2 * (s - 2))
                    k0 = (t0 - sum(TP[:s])) // 2
                    for i in range(np_):
                        kk = k0 + i
                        mm = pe.matmul(pb[s % 3][:, 0:D], id2_sb[:, :, :],
                                       xbuf[slot_of[g]][:, 2 * i:2 * i + 2, :],
                                       start=(kk == 0), stop=(kk == PAIRS[s] - 1),
                                       perf_mode=DR)
                        if i == np_ - 1:
                            mm.then_inc(s_pe, 1)
                    for (fn, w, h) in stage_after.get(g, []):
                        fn(pe, w, h)
                # tail: window 3 zippered
                p = NSLOT - 1
                for fn in (mlp_tr, mlp_z, mlp_h, mlp_o):
                    fn(pe, p, 0)
                    fn(pe, p, 1)

            # ---- DVE-side drains, stage granular
            def dve_ptc(v, w, h):
                q, cp, wch = _chunk(w, h)
                v.wait_ge(s_tr, 2 * q + 2)
                for k in range(2):
                    v.tensor_copy(pT[k][:, wch],
                                  A[cp][:, k * 64:(k + 1) * 64]).then_inc(s_ptc, 1)

            def dve_zc(v, w, h):
                q, cp, wch = _chunk(w, h)
                if q == 0:
                    v.wait_ge(s_ce, 64)   # binT/b1T/b2T/cbc loaded
                v.wait_ge(s_z, 2 * q + 2)
                for j in range(2):
                    # zT = zP + b_in[j-block] (x) counts
                    v.scalar_tensor_tensor(
                        zT[j][:, wch], cbc[:, wch], binT[:, j:j + 1],
                        A[cp][:, 128 + j * 64:128 + (j + 1) * 64],
                        MULT, ADD).then_inc(s_zc, 1)

            def dve_hc(v, w, h):
                q, cp, wch = _chunk(w, h)
                v.wait_ge(s_h, 4 * q + 4)
                for j in range(4):
                    # hT = relu(hP + b1[j-block])
                    v.tensor_scalar(hT[j][:, wch],
                                    hB[cp][:, j * 64:(j + 1) * 64],
                                    b1T[:, j:j + 1], 0.0, ADD, MAX).then_inc(s_hc, 1)

            def dve_oc(v, w, h):
                q, cp, wch = _chunk(w, h)
                v.wait_ge(s_o, 2 * q + 2)
                for j in range(2):
                    # ot = oP + b2[j-block]
                    v.tensor_scalar(ot[j][:, wch],
                                    A[cp][:, 256 + j * 64:256 + (j + 1) * 64],
                                    b2T[:, j:j + 1], None, ADD).then_inc(s_oc, 1)

            @block.vector
            def _(v):
                # drains for window w-1 that gate PE's pre-flush stages MUST
                # precede flush(w); hc(w-1,1)/oc(w-1,*) only gate stages PE
                # reaches after flush(w), so flush slots in between (it waits
                # only on s_pe, which PE raises before those stages).
                def flush(w):
                    v.wait_ge(s_pe, cumslabs[w])
                    for k in range(2):
                        v.tensor_copy(po[w][:, k * 128:(k + 1) * 128],
                                      pb[w % 3][:, k * 128:(k + 1) * 128]
                                      ).then_inc(s_fl, 1)
                flush(0)
                for w in range(1, NSLOT + 1):
                    p = w - 1
                    dve_ptc(v, p, 0)
                    dve_ptc(v, p, 1)
                    dve_zc(v, p, 0)
                    dve_zc(v, p, 1)
                    dve_hc(v, p, 0)
                    if w < NSLOT:
                        flush(w)
                    dve_hc(v, p, 1)
                    dve_oc(v, p, 0)
                    dve_oc(v, p, 1)

    return nc


def _quantize_feedback(x, sizes, starts, order):
    """fp8 e4m3 with per-(segment, column) sigma-delta error feedback."""
    xq = np.empty(x.shape, dtype=NPF8)
    # process segments in descending-size order so live set is a prefix
    sz_d = sizes[order]                       # descending
    st_d = starts[order]
    carry = np.zeros((NSEG, D), np.float32)
    maxlen = int(sz_d[0])
    for r in range(maxlen):
        m = int(np.searchsorted(-sz_d, -(r + 1), side="right"))
        rows = st_d[:m] + r
        acc = x[rows] + carry[:m]
        q = acc.astype(NPF8)
        xq[rows] = q
        carry[:m] = acc - q.astype(np.float32)
    return xq


def _plan(batch):
    sizes = np.bincount(batch, minlength=NSEG).astype(np.int64)
    starts = np.concatenate([[0], np.cumsum(sizes)])[:-1]
    order = np.argsort(-sizes, kind="stable")
    TP = [int(sizes[order[1024 * s]] + 1) // 2 * 2 for s in range(NSLOT)]
    return sizes, starts, order, TP


def prepare_inputs(inputs):
    """Host-side shard plan: returns (TP, per_core input maps, core_segs)."""
    x = np.ascontiguousarray(np.asarray(inputs["x"], np.float32))
    batch = np.asarray(inputs["batch"]).astype(np.int64)
    W_in = np.asarray(inputs["W_in"], np.float32)
    b_in = np.asarray(inputs["b_in"], np.float32).reshape(1, D)
    W1 = np.asarray(inputs["W1"], np.float32)
    b1 = np.asarray(inputs["b1"], np.float32).reshape(1, 2 * D)
    W2 = np.asarray(inputs["W2"], np.float32)
    b2 = np.asarray(inputs["b2"], np.float32).reshape(1, D)

    sizes, starts, order, TP = _plan(batch)
    TOT = sum(TP)
    xq = _quantize_feedback(x, sizes, starts, order)
    xq_pad = np.concatenate([xq, np.zeros((1, D), NPF8)])

    id2 = np.stack([np.eye(128, dtype=np.float32)] * 2, axis=1).astype(NPF8)
    idf = np.eye(128, dtype=np.float32)
    shared = dict(
        id2=id2, idf=idf,
        win=W_in.astype(NPBF), w1=W1.astype(NPBF), w2=W2.astype(NPBF),
        binT=np.ascontiguousarray(b_in.reshape(2, 128).T),
        b1T=np.ascontiguousarray(b1.reshape(4, 128).T),
        b2T=np.ascontiguousarray(b2.reshape(2, 128).T),
    )

    per_core = []
    core_segs = []
    for c in range(N_CORES):
        idx = np.full((128, TOT), N, np.int64)
        segs_c = np.empty(SEG, np.int64)
        off = 0
        for s in range(NSLOT):
            w = 8 * s + c
            segs = order[128 * w:128 * w + 128]
            segs_c[128 * s:128 * s + 128] = segs
            for p in range(128):
                n = int(sizes[segs[p]])
                idx[p, off:off + n] = starts[segs[p]] + np.arange(n)
            off += TP[s]
        xdr = xq_pad[idx.reshape(-1)].reshape(128, TOT, D)
        crow = sizes[segs_c].astype(np.float32).reshape(1, SEG)
        m = dict(shared)
        m.update(xdr=xdr, cbc=np.repeat(crow, 128, axis=0))
        per_core.append(m)
        core_segs.append(segs_c)
    return TP, per_core, core_segs


def kernel(**inputs):
    TP, per_core, core_segs = prepare_inputs(inputs)
    nc = build_program(TP)
    res = run_bass_kernel_spmd(nc, per_core, list(range(N_CORES)))

    out = np.empty((NSEG, D), np.float32)
    for c in range(N_CORES):
        out[core_segs[c]] = res.results[c]["outT"].T
    return out
